# revision 2
# baseline (speedup 1.0000x reference)
"""DirectedGraphConvolution Trainium2 kernel (bf16 restructure).

Per batch element b (one per NeuronCore, 8 total, data-parallel):
    N_e = H @ W                          [n, dout]
    T1  = G  @ N_e ; T2 = G.T @ N_e
    rs  = G.sum(-1); cs = G.sum(-2)
    out = [ relu(0.5*(T1 + T2)),
            relu(G.T @ (T1 / rs[:,None])),
            relu(G  @ (T2 / cs[:,None])) ]

Schedule (v2): everything in bf16 (PSUM accumulation f32).  G streams in
f32, is cast to bf16 (ACT) into persistent natural tiles g16, and each
arriving row-tile is immediately transposed on PE (bf16, 1 cyc/row) into
persistent G.T strips.  T1 = G @ N_e (pass B1) runs per-tile during the
DMA window using the fresh strips, so the PE is busy while G loads.
Post-load: pass A (T2 = G.T @ N_e, g16 stationaries) emits out1 +
T2/cs early; pass C (out2 = G.T @ T1') and pass B2 (out3 = G @ T2',
strips stationaries -- no second transpose pass).  rs reduces from g16
on DVE; cs reduces from the strips.
"""

import numpy as np
import concourse.bass as bass
import concourse.mybir as mybir
import concourse.tile as tile
from concourse import bacc
from concourse.bass_utils import run_bass_kernel_spmd
from concourse.masks import make_identity

F32 = mybir.dt.float32
BF16 = mybir.dt.bfloat16
RELU = mybir.ActivationFunctionType.Relu
AX = mybir.AxisListType.X

P = 128
B = 8
N = 2048
NO = N // P            # 16 row tiles
DIN = 256
DOUT = 256
KO = DIN // P          # 2 k tiles for H @ W
W3 = 3 * DOUT


def build():
    nc = bacc.Bacc("TRN2", target_bir_lowering=False)
    G = nc.declare_dram_parameter("G", [N, N], F32, isOutput=False)
    H = nc.declare_dram_parameter("H", [N, DIN], F32, isOutput=False)
    W = nc.declare_dram_parameter("W", [DIN, DOUT], F32, isOutput=False)
    out = nc.declare_dram_parameter("out", [N, W3], F32, isOutput=True)

    G_r = G.rearrange("(o p) j -> p o j", p=P)
    H_r = H.rearrange("(o p) d -> p o d", p=P)
    W_r = W.rearrange("(o p) d -> p o d", p=P)
    out_r = out.rearrange("(o p) d -> p o d", p=P)

    with tile.TileContext(nc) as tc:
        with (
            tc.tile_pool(name="const", bufs=1) as const,
            tc.tile_pool(name="gn", bufs=1) as gn,
            tc.tile_pool(name="gt", bufs=1) as gt,
            tc.tile_pool(name="gstage", bufs=2) as gstage,
            tc.tile_pool(name="nep", bufs=1) as nep,
            tc.tile_pool(name="t1fp", bufs=1) as t1fp,
            tc.tile_pool(name="t1pp", bufs=1) as t1pp,
            tc.tile_pool(name="t2pp", bufs=1) as t2pp,
            tc.tile_pool(name="stage", bufs=4) as stage,
        ):
            # ---- constants ----
            ident_f32 = const.tile([P, P], F32)
            make_identity(nc, ident_f32)
            ident16 = const.tile([P, P], BF16)
            nc.vector.tensor_copy(ident16, ident_f32)
            rsinv = const.tile([P, NO, 1], F32)
            csinv = const.tile([P, NO, 1], F32)

            # ---- persistent bf16 tensors ----
            g16 = [gn.tile([P, N], BF16, tag=f"g{o}", name=f"g{o}") for o in range(NO)]
            strips = [
                gt.tile([P, N], BF16, tag=f"s{j}", name=f"s{j}") for j in range(NO)
            ]
            ne16 = nep.tile([P, NO, DOUT], BF16)
            t1f = t1fp.tile([P, NO, DOUT], BF16)
            t1p = t1pp.tile([P, NO, DOUT], BF16)
            t2p = t2pp.tile([P, NO, DOUT], BF16)

            # ---- input DMAs ----
            # W + H on the scalar (ACT) queue: issued up front, before the
            # ACT engine gets busy casting G.  G on the sync (SP) queue.
            w_st = const.tile([P, KO, DOUT], F32)
            nc.scalar.dma_start(w_st, W_r)
            w16 = const.tile([P, KO, DOUT], BF16)
            nc.vector.tensor_copy(w16, w_st)

            gs_tiles = []
            for o in range(NO):
                gs = gstage.tile([P, N], F32, tag="gs", name=f"gs{o}")
                nc.sync.dma_start(gs[:, 0:N // 2], G_r[:, o, 0:N // 2])
                nc.sync.dma_start(gs[:, N // 2:N], G_r[:, o, N // 2:N])
                gs_tiles.append(gs)

            # ---- H @ W -> ne16 (bf16 pipeline, PE transposes H blocks) ----
            with (
                tc.tile_pool(name="hio", bufs=4) as hio,
                tc.tile_pool(name="h16p", bufs=3) as h16p,
                tc.tile_pool(name="htp", bufs=3) as htp,
                tc.tile_pool(name="ps_h", bufs=1, space="PSUM") as ps_h,
                tc.tile_pool(name="ps_ne", bufs=2, space="PSUM") as ps_ne,
            ):
                psh = ps_h.tile([P, 8, P], BF16)  # 8 transpose slots = 1 bank
                hts = {}
                h16s = {}
                for t in range(NO + 1):
                    if t < NO:
                        hst = hio.tile([P, DIN], F32, tag="hr")
                        nc.scalar.dma_start(hst, H_r[:, t, :])
                        h16_t = h16p.tile([P, DIN], BF16, tag="h16")
                        nc.scalar.copy(h16_t, hst)
                        h16s[t] = h16_t
                        ht_t = htp.tile([P, KO, P], BF16, tag="ht")
                        for kt in range(KO):
                            s = (2 * t + kt) % 8
                            nc.tensor.transpose(
                                psh[:, s, :], h16_t[:, kt * P:(kt + 1) * P], ident16
                            )
                            nc.vector.tensor_copy(ht_t[:, kt, :], psh[:, s, :])
                        hts[t] = ht_t
                    if t >= 1:
                        u = t - 1
                        ht_u = hts.pop(u)
                        h16s.pop(u)
                        pne = ps_ne.tile([P, DOUT], F32, tag="pne")
                        for kt in range(KO):
                            nc.tensor.matmul(
                                pne,
                                ht_u[:, kt, :],
                                w16[:, kt, :],
                                start=(kt == 0),
                                stop=(kt == KO - 1),
                            )
                        nc.vector.tensor_copy(ne16[:, u, :], pne)

            # ---- phase L: per arriving G tile: cast, rs, transposes, B1 ----
            with (
                tc.tile_pool(name="psT", bufs=1, space="PSUM") as psTp,
                tc.tile_pool(name="psB1", bufs=2, space="PSUM") as psB1,
                tc.tile_pool(name="tmpp", bufs=2) as tmpp,
            ):
                psT = psTp.tile([P, 8, P], BF16)  # 8 transpose slots = 1 bank
                for o in range(NO):
                    nc.scalar.copy(g16[o], gs_tiles[o])
                    rs_t = tmpp.tile([P, 1], F32, tag="rs")
                    nc.vector.reduce_sum(rs_t, g16[o], axis=AX)
                    nc.vector.reciprocal(rsinv[:, o, :], rs_t)

                    pb1 = psB1.tile([P, DOUT], F32, tag="pb1")
                    LOOK = 3
                    for step in range(NO + LOOK):
                        if step < NO:
                            jt = step
                            s = jt % 8
                            nc.tensor.transpose(
                                psT[:, s, :],
                                g16[o][:, jt * P:(jt + 1) * P],
                                ident16,
                            )
                            dst = strips[jt][:, o * P:(o + 1) * P]
                            if jt % 8 < 5:
                                nc.vector.tensor_copy(dst, psT[:, s, :])
                            else:
                                nc.scalar.copy(dst, psT[:, s, :])
                        if step >= LOOK:
                            jm = step - LOOK
                            nc.tensor.matmul(
                                pb1,
                                strips[jm][:, o * P:(o + 1) * P],
                                ne16[:, jm, :],
                                start=(jm == 0),
                                stop=(jm == NO - 1),
                            )
                    nc.vector.tensor_copy(t1f[:, o, :], pb1)
                    nc.vector.tensor_scalar_mul(t1p[:, o, :], pb1, rsinv[:, o, 0:1])

            # ---- pass A: T2 = G.T @ N_e ; out1, T2' ----
            with (
                tc.tile_pool(name="psA", bufs=4, space="PSUM") as psA,
                tc.tile_pool(name="tmpa", bufs=3) as tmpa,
            ):
                for jt in range(NO):
                    cs_t = tmpa.tile([P, 1], F32, tag="cs")
                    nc.vector.reduce_sum(cs_t, strips[jt], axis=AX)
                    nc.vector.reciprocal(csinv[:, jt, :], cs_t)
                    pa = psA.tile([P, DOUT], F32, tag="pa")
                    for kt in range(NO):
                        nc.tensor.matmul(
                            pa,
                            g16[kt][:, jt * P:(jt + 1) * P],
                            ne16[:, kt, :],
                            start=(kt == 0),
                            stop=(kt == NO - 1),
                        )
                    nc.vector.tensor_scalar_mul(t2p[:, jt, :], pa, csinv[:, jt, 0:1])
                    o1t = tmpa.tile([P, DOUT], F32, tag="o1t")
                    nc.vector.tensor_add(o1t, pa, t1f[:, jt, :])
                    o1 = stage.tile([P, DOUT], F32, tag="o1")
                    nc.scalar.activation(o1, o1t, RELU, scale=0.5)
                    nc.sync.dma_start(out_r[:, jt, 0:DOUT], o1)

            # ---- pass C: out2 = relu(G.T @ T1') ----
            with tc.tile_pool(name="psC", bufs=3, space="PSUM") as psC:
                for jt in range(NO):
                    pc = psC.tile([P, DOUT], F32, tag="pc")
                    for kt in range(NO):
                        nc.tensor.matmul(
                            pc,
                            g16[kt][:, jt * P:(jt + 1) * P],
                            t1p[:, kt, :],
                            start=(kt == 0),
                            stop=(kt == NO - 1),
                        )
                    o2 = stage.tile([P, DOUT], F32, tag="o2")
                    nc.scalar.activation(o2, pc, RELU)
                    nc.sync.dma_start(out_r[:, jt, DOUT:2 * DOUT], o2)

            # ---- pass B2: out3 = relu(G @ T2') ----
            with tc.tile_pool(name="psB2", bufs=3, space="PSUM") as psB2:
                for it in range(NO):
                    pb = psB2.tile([P, DOUT], F32, tag="pb")
                    for jt in range(NO):
                        nc.tensor.matmul(
                            pb,
                            strips[jt][:, it * P:(it + 1) * P],
                            t2p[:, jt, :],
                            start=(jt == 0),
                            stop=(jt == NO - 1),
                        )
                    o3 = stage.tile([P, DOUT], F32, tag="o3")
                    nc.scalar.activation(o3, pb, RELU)
                    nc.sync.dma_start(out_r[:, it, 2 * DOUT:W3], o3)

    nc.compile()
    return nc


_NC = None


def _get_nc():
    global _NC
    if _NC is None:
        _NC = build()
    return _NC


def run(inputs: dict, trace: bool = False):
    """Run on 8 cores; returns (stacked_out [B,N,W3], BassKernelResults)."""
    H, G, W = inputs["H"], inputs["G"], inputs["W"]
    H = np.ascontiguousarray(H, dtype=np.float32)
    G = np.ascontiguousarray(G, dtype=np.float32)
    W = np.ascontiguousarray(W, dtype=np.float32)
    in_maps = [
        {"G": np.ascontiguousarray(G[b]), "H": np.ascontiguousarray(H[b]), "W": W}
        for b in range(B)
    ]
    nc = _get_nc()
    res = run_bass_kernel_spmd(nc, in_maps, core_ids=list(range(B)), trace=trace)
    out = np.stack([res.results[b]["out"] for b in range(B)], axis=0)
    return out, res


def kernel(H, G, W):
    out, _ = run({"H": H, "G": G, "W": W})
    return out


# revision 7
# speedup vs baseline: 1.5990x; 1.5990x over previous
"""DirectedGraphConvolution Trainium2 kernel (bf16, v3).

Per batch element b (one per NeuronCore, 8 total, data-parallel):
    N_e = H @ W                          [n, dout]
    T1  = G  @ N_e ; T2 = G.T @ N_e
    rs  = G.sum(-1); cs = G.sum(-2)
    out = [ relu(0.5*(T1 + T2)),
            relu(G.T @ (T1 / rs[:,None])),
            relu(G  @ (T2 / cs[:,None])) ]

Schedule: all matmuls bf16 (PSUM f32).  G streams in f32 (sync queue,
3-deep ring) and is cast to bf16 on ACT -- the cast's accum_out gives
the row sums rs for free.  Each arriving row-tile is transposed on PE
(bf16, 1 cyc/row) into a persistent G.T strip tensor via strided quad
copies (DVE/GpSimd), and B1 = G @ N_e runs one tile behind, so the PE
stays busy through the whole DMA window.  Post-load, passes A and C
share their stationaries (natural G blocks), so they run as ONE pass
with moving [N_e | T1'] (512 wide): T2, out1, out2 per column tile,
with cs from GpSimd strip reductions.  B2 = G @ T2' reuses the strips
(no second transpose pass).  Outputs stream per-tile on idle queues.
"""

import numpy as np
import concourse.bass as bass
import concourse.mybir as mybir
import concourse.tile as tile
from concourse import bacc
from concourse.bass_utils import run_bass_kernel_spmd
from concourse.masks import make_identity

F32 = mybir.dt.float32
BF16 = mybir.dt.bfloat16
COPY = mybir.ActivationFunctionType.Copy
RELU = mybir.ActivationFunctionType.Relu
AX = mybir.AxisListType.X

P = 128
B = 8
N = 2048
NO = N // P            # 16 row tiles
DIN = 256
DOUT = 256
KO = DIN // P          # 2 k tiles for H @ W
W3 = 3 * DOUT


def build():
    nc = bacc.Bacc("TRN2", target_bir_lowering=False)
    G = nc.declare_dram_parameter("G", [N, N], F32, isOutput=False)
    H = nc.declare_dram_parameter("H", [N, DIN], F32, isOutput=False)
    W = nc.declare_dram_parameter("W", [DIN, DOUT], F32, isOutput=False)
    out = nc.declare_dram_parameter("out", [N, W3], F32, isOutput=True)

    G_r = G.rearrange("(o p) j -> p o j", p=P)
    H_r = H.rearrange("(o p) d -> p o d", p=P)
    W_r = W.rearrange("(o p) d -> p o d", p=P)
    out_r = out.rearrange("(o p) d -> p o d", p=P)

    with tile.TileContext(nc) as tc:
        with (
            tc.tile_pool(name="const", bufs=1) as const,
            tc.tile_pool(name="gn", bufs=1) as gn,
            tc.tile_pool(name="gt", bufs=1) as gt,
            tc.tile_pool(name="gstage", bufs=3) as gstage,
            tc.tile_pool(name="nmp", bufs=1) as nmp,
            tc.tile_pool(name="t1fp", bufs=1) as t1fp,
            tc.tile_pool(name="t2pp", bufs=1) as t2pp,
            tc.tile_pool(name="stage", bufs=3) as stage,
        ):
            # ---- constants ----
            ident_f32 = const.tile([P, P], F32)
            make_identity(nc, ident_f32)
            ident16 = const.tile([P, P], BF16)
            nc.vector.tensor_copy(ident16, ident_f32)
            rsinv = const.tile([P, NO, 1], F32)
            csinv = const.tile([P, NO, 1], F32)

            # ---- persistent bf16 tensors ----
            g16 = [gn.tile([P, N], BF16, tag=f"g{o}", name=f"g{o}") for o in range(NO)]
            strips = gt.tile([P, NO, N], BF16)     # [col-in-block, jt, row]
            # moving operand per k tile: [N_e (256) | ones (2) | T1' (256) | pad]
            nm = nmp.tile([P, NO, 516], BF16)
            nc.vector.memset(nm[:, :, DOUT:DOUT + 2], 1.0)
            t1f = t1fp.tile([P, NO, DOUT], BF16)
            t2p = t2pp.tile([P, NO, DOUT], BF16)

            # ---- input DMAs ----
            w_st = const.tile([P, KO, DOUT], F32)
            nc.scalar.dma_start(w_st, W_r)
            w16 = const.tile([P, KO, DOUT], BF16)
            nc.vector.tensor_copy(w16, w_st)

            gs_tiles = []
            for o in range(NO):
                gs = gstage.tile([P, N], F32, tag="gs", name=f"gs{o}")
                nc.sync.dma_start(gs[:, 0:N // 2], G_r[:, o, 0:N // 2])
                nc.sync.dma_start(gs[:, N // 2:N], G_r[:, o, N // 2:N])
                gs_tiles.append(gs)

            # ---- H @ W -> nm[:, :, 0:DOUT] (bf16, PE transposes H blocks) ----
            with (
                tc.tile_pool(name="hio", bufs=4) as hio,
                tc.tile_pool(name="h16p", bufs=3) as h16p,
                tc.tile_pool(name="htp", bufs=3) as htp,
                tc.tile_pool(name="ps_h", bufs=1, space="PSUM") as ps_h,
                tc.tile_pool(name="ps_ne", bufs=2, space="PSUM") as ps_ne,
            ):
                psh = ps_h.tile([P, 8, P], BF16)  # 8 transpose slots = 1 bank
                hts = {}
                for t in range(NO + 1):
                    if t < NO:
                        hst = hio.tile([P, DIN], F32, tag="hr")
                        nc.scalar.dma_start(hst, H_r[:, t, :])
                        h16_t = h16p.tile([P, DIN], BF16, tag="h16")
                        nc.scalar.copy(h16_t, hst)
                        ht_t = htp.tile([P, KO, P], BF16, tag="ht")
                        for kt in range(KO):
                            s = (2 * t + kt) % 8
                            nc.tensor.transpose(
                                psh[:, s, :], h16_t[:, kt * P:(kt + 1) * P], ident16
                            )
                            nc.vector.tensor_copy(ht_t[:, kt, :], psh[:, s, :])
                        hts[t] = ht_t
                    if t >= 1:
                        u = t - 1
                        ht_u = hts.pop(u)
                        pne = ps_ne.tile([P, DOUT], F32, tag="pne")
                        for kt in range(KO):
                            nc.tensor.matmul(
                                pne,
                                ht_u[:, kt, :],
                                w16[:, kt, :],
                                start=(kt == 0),
                                stop=(kt == KO - 1),
                            )
                        nc.vector.tensor_copy(nm[:, u, 0:DOUT], pne)

            # ---- phase L: per arriving G tile: cast(+rs), transposes, B1 ----
            with (
                tc.tile_pool(name="psT", bufs=1, space="PSUM") as psTp,
                tc.tile_pool(name="psB1", bufs=2, space="PSUM") as psB1,
                tc.tile_pool(name="tmpp", bufs=2) as tmpp,
            ):
                psT = psTp.tile([P, 8, P], BF16)  # 8 transpose slots = 1 bank

                def b1_pass(u):
                    # [T1 | rs] = G[u rows, :] @ [N_e | ones]
                    pb1 = psB1.tile([P, DOUT + 2], F32, tag="pb1")
                    for jm in range(NO):
                        nc.tensor.matmul(
                            pb1,
                            strips[:, jm, u * P:(u + 1) * P],
                            nm[:, jm, 0:DOUT + 2],
                            start=(jm == 0),
                            stop=(jm == NO - 1),
                        )
                    # t1f = T1 (bf16), nm[.., 258:514] = T1' = T1 * rsinv
                    nc.scalar.activation(t1f[:, u, :], pb1[:, 0:DOUT], COPY)
                    nc.vector.reciprocal(rsinv[:, u, :], pb1[:, DOUT:DOUT + 1])
                    nc.vector.tensor_scalar_mul(
                        nm[:, u, DOUT + 2:DOUT + 2 + DOUT],
                        pb1[:, 0:DOUT],
                        rsinv[:, u, 0:1],
                    )

                for o in range(NO):
                    nc.gpsimd.tensor_copy(g16[o], gs_tiles[o])
                    for q in range(4):       # 4 transposes + 1 quad copy, x4
                        for jt in range(4 * q, 4 * q + 4):
                            nc.tensor.transpose(
                                psT[:, jt % 8, :],
                                g16[o][:, jt * P:(jt + 1) * P],
                                ident16,
                            )
                        src = psT[:, (4 * q) % 8:(4 * q) % 8 + 4, :]
                        dst = strips[:, 4 * q:4 * q + 4, o * P:(o + 1) * P]
                        if q % 2 == 0:
                            nc.vector.tensor_copy(dst, src)
                        else:
                            nc.scalar.copy(dst, src)
                    if o >= 1:
                        b1_pass(o - 1)
                b1_pass(NO - 1)

            # ---- pass A: [T2 | cs] = G.T @ [N_e | ones]; out1, T2' ----
            with (
                tc.tile_pool(name="psA", bufs=4, space="PSUM") as psA,
                tc.tile_pool(name="tmpa", bufs=3) as tmpa,
            ):
                for jt in range(NO):
                    pa = psA.tile([P, DOUT + 2], F32, tag="pa")
                    for kt in range(NO):
                        nc.tensor.matmul(
                            pa,
                            g16[kt][:, jt * P:(jt + 1) * P],
                            nm[:, kt, 0:DOUT + 2],
                            start=(kt == 0),
                            stop=(kt == NO - 1),
                        )
                    nc.vector.reciprocal(csinv[:, jt, :], pa[:, DOUT:DOUT + 1])
                    nc.vector.tensor_scalar_mul(
                        t2p[:, jt, :], pa[:, 0:DOUT], csinv[:, jt, 0:1]
                    )
                    o1t = tmpa.tile([P, DOUT], F32, tag="o1t")
                    nc.vector.tensor_add(o1t, pa[:, 0:DOUT], t1f[:, jt, :])
                    o1 = stage.tile([P, DOUT], F32, tag="o1")
                    nc.scalar.activation(o1, o1t, RELU, scale=0.5)
                    nc.gpsimd.dma_start(out_r[:, jt, 0:DOUT], o1)

            # ---- pass C: out2 = relu(G.T @ T1') ----
            with tc.tile_pool(name="psC", bufs=3, space="PSUM") as psC:
                for jt in range(NO):
                    pc = psC.tile([P, DOUT], F32, tag="pc")
                    for kt in range(NO):
                        nc.tensor.matmul(
                            pc,
                            g16[kt][:, jt * P:(jt + 1) * P],
                            nm[:, kt, DOUT + 2:DOUT + 2 + DOUT],
                            start=(kt == 0),
                            stop=(kt == NO - 1),
                        )
                    o2 = stage.tile([P, DOUT], F32, tag="o2")
                    nc.scalar.activation(o2, pc, RELU)
                    nc.sync.dma_start(out_r[:, jt, DOUT:2 * DOUT], o2)

            # ---- pass B2: out3 = relu(G @ T2') ----
            with tc.tile_pool(name="psB2", bufs=3, space="PSUM") as psB2:
                for it in range(NO):
                    pb = psB2.tile([P, DOUT], F32, tag="pb")
                    for jt in range(NO):
                        nc.tensor.matmul(
                            pb,
                            strips[:, jt, it * P:(it + 1) * P],
                            t2p[:, jt, :],
                            start=(jt == 0),
                            stop=(jt == NO - 1),
                        )
                    o3 = stage.tile([P, DOUT], F32, tag="o3")
                    nc.scalar.activation(o3, pb, RELU)
                    nc.sync.dma_start(out_r[:, it, 2 * DOUT:W3], o3)

    nc.compile()
    return nc


_NC = None


def _get_nc():
    global _NC
    if _NC is None:
        _NC = build()
    return _NC


def run(inputs: dict, trace: bool = False):
    """Run on 8 cores; returns (stacked_out [B,N,W3], BassKernelResults)."""
    H, G, W = inputs["H"], inputs["G"], inputs["W"]
    H = np.ascontiguousarray(H, dtype=np.float32)
    G = np.ascontiguousarray(G, dtype=np.float32)
    W = np.ascontiguousarray(W, dtype=np.float32)
    in_maps = [
        {"G": np.ascontiguousarray(G[b]), "H": np.ascontiguousarray(H[b]), "W": W}
        for b in range(B)
    ]
    nc = _get_nc()
    res = run_bass_kernel_spmd(nc, in_maps, core_ids=list(range(B)), trace=trace)
    out = np.stack([res.results[b]["out"] for b in range(B)], axis=0)
    return out, res


def kernel(H, G, W):
    out, _ = run({"H": H, "G": G, "W": W})
    return out
